# revision 9
# baseline (speedup 1.0000x reference)
"""GCN2 (GCNII) forward pass on 8 Trainium2 NeuronCores via Bass/Tile.

v2 strategy (node sharding, on-chip selection-matrix generation):
  - dst nodes sharded across 8 cores; per-core slice padded to NLOC rows.
  - full h replica rebuilt each layer via AllGather (bf16, DRAM Shared).
  - aggregation: edges grouped by (dst group of 1024, 2-core src bucket).
    Per (group, bucket) ONE dma_gather pulls the needed h rows (int16
    window indices, trailing -1 pads are skipped by the ucode); selection
    matrices are GENERATED ON-CHIP (DVE iota==slot * w) from compact
    per-edge (slot, weight) tables resident in SBUF — no S streaming.
  - x0 residual is folded into PSUM via identity matmuls (also serves as
    the PSUM init for every 128-dst block).
  - epilogue applies Wt = (1-beta)I + beta*W per 128-node block, relu,
    writes the new h slice; AllGather rebuilds the replica.
  - last layer computes logits + log_softmax on-chip, f32 output.
"""
import math
import numpy as np

import concourse.bass as bass
import concourse.bacc as bacc
import concourse.tile as tile
import concourse.mybir as mybir
import concourse.bass_utils as bass_utils
from concourse.masks import make_identity


# ---------------- configuration ----------------
class CFG:
    NC = 8
    N = 100000
    F_IN = 512
    H = 128
    C = 40
    L = 8
    ALPHA = 0.1
    THETA = 0.5
    GRP = 1024
    BLK = 128
    NBUCK = 4          # src buckets of 2 cores each (int16 window limit)
    MAIN_COLS_MAX = 4  # max 128-edge columns per (block, bucket) cell
    OVF_COLS = 2       # group-wide overflow columns per (group, bucket)

    @classmethod
    def derived(cls):
        cls.NPC = cls.N // cls.NC          # real rows per core
        cls.NGRP = math.ceil(cls.NPC / cls.GRP)
        cls.NLOC = cls.NGRP * cls.GRP      # padded rows per core
        cls.NBLK = cls.GRP // cls.BLK
        cls.NGJ = cls.NGRP * cls.NBUCK
        cls.WIN = 2 * cls.NLOC             # gather window rows (2 cores)
        cls.KT = cls.F_IN // 128
        cls.NBLK_TOT = cls.NGRP * cls.NBLK


CFG.derived()


# ---------------- host preprocessing ----------------
def preprocess(edge_index, edge_weight):
    """Shared program structure + per-core device arrays.

    Structure (identical on all cores, derived from max counts):
      ncols[gj], col2b[gj] (block of each main column), num_idxs[gj].
    Per-core: wrapped int16 idx table, compact fp16 slot / bf16 weight
    tables for on-chip selection-matrix generation.
    """
    c_ = CFG
    src_a = edge_index[0].astype(np.int64)
    dst_a = edge_index[1].astype(np.int64)
    w_a = edge_weight.astype(np.float32) * (1.0 - c_.ALPHA)

    core_of = dst_a // c_.NPC
    dl_a = dst_a - core_of * c_.NPC
    g_a = dl_a // c_.GRP
    b_a = (dl_a % c_.GRP) // c_.BLK
    j2_a = src_a // (2 * c_.NPC)
    jcore = src_a // c_.NPC
    ilocal = src_a - jcore * c_.NPC
    idxrow_a = (jcore - 2 * j2_a) * c_.NLOC + ilocal
    s128_a = dl_a % c_.BLK
    s1024_a = dl_a % c_.GRP

    # shared cols per cell from max counts over cores
    cellkey = ((core_of * c_.NGRP + g_a) * c_.NBUCK + j2_a) * c_.NBLK + b_a
    counts = np.bincount(cellkey, minlength=c_.NC * c_.NGRP * c_.NBUCK * c_.NBLK)
    counts = counts.reshape(c_.NC, c_.NGRP, c_.NBUCK, c_.NBLK)
    cmax = counts.max(axis=0)  # [NGRP, NBUCK, NBLK]
    cols_cell = np.minimum(c_.MAIN_COLS_MAX, -(-cmax // 128))  # ceil div

    ncols = []      # per gj: number of main columns
    col2b = []      # per gj: block index of each main column
    num_idxs = []   # per gj: static gather count (includes ovf cols)
    for g in range(c_.NGRP):
        for j2 in range(c_.NBUCK):
            cc = cols_cell[g, j2]
            ncols.append(int(cc.sum()))
            cb = []
            for b in range(c_.NBLK):
                cb += [b] * int(cc[b])
            col2b.append(cb)
            num_idxs.append((int(cc.sum()) + c_.OVF_COLS) * 128)
    idx_len = [n for n in num_idxs]
    idx_off = np.concatenate([[0], np.cumsum(idx_len)]).astype(np.int64)
    nsc = [ncols[gj] + c_.OVF_COLS for gj in range(c_.NGJ)]
    sc_off = np.concatenate([[0], np.cumsum(nsc)]).astype(np.int64)
    tot_idx = int(idx_off[-1])
    tot_sc = int(sc_off[-1])

    structure = dict(ncols=ncols, col2b=col2b, num_idxs=num_idxs,
                     idx_off=idx_off, sc_off=sc_off,
                     tot_idx=tot_idx, tot_sc=tot_sc)

    cores = []
    for c in range(c_.NC):
        m = core_of == c
        g_c, b_c, j2_c = g_a[m], b_c_ln(b_a, m), j2_a[m]
        ir_c, s128_c, s1024_c, w_c = idxrow_a[m], s128_a[m], s1024_a[m], w_a[m]
        order = np.lexsort((ir_c, b_c, j2_c, g_c))
        g_c, b_c, j2_c = g_c[order], b_c[order], j2_c[order]
        ir_c, s128_c, s1024_c, w_c = ir_c[order], s128_c[order], s1024_c[order], w_c[order]

        key = (g_c * c_.NBUCK + j2_c) * c_.NBLK + b_c
        seg0 = np.searchsorted(key, np.arange(c_.NGJ * c_.NBLK), side="left")
        seg1 = np.searchsorted(key, np.arange(c_.NGJ * c_.NBLK), side="right")

        idx_flat = np.zeros(tot_idx, np.int16)
        slot_t = np.full((128, tot_sc), -1.0, np.float32)
        w_t = np.zeros((128, tot_sc), np.float32)

        for gj in range(c_.NGJ):
            io = idx_off[gj]
            so = sc_off[gj]
            nc_gj = ncols[gj]
            ovf_ir, ovf_s1024, ovf_w = [], [], []
            coff = 0
            for b in range(c_.NBLK):
                ci = gj * c_.NBLK + b
                s0, s1 = seg0[ci], seg1[ci]
                ncb = col2b[gj].count(b)
                cap = ncb * 128
                n_here = s1 - s0
                take = min(n_here, cap)
                if take > 0:
                    pos = np.arange(take)
                    idx_flat[io + (coff * 128) + pos] = ir_c[s0:s0 + take]
                    slot_t[pos % 128, so + coff + pos // 128] = s128_c[s0:s0 + take]
                    w_t[pos % 128, so + coff + pos // 128] = w_c[s0:s0 + take]
                if n_here > cap:
                    ovf_ir.append(ir_c[s0 + cap:s1])
                    ovf_s1024.append(s1024_c[s0 + cap:s1])
                    ovf_w.append(w_c[s0 + cap:s1])
                coff += ncb
            # merged overflow, sorted by idxrow
            if ovf_ir:
                o_ir = np.concatenate(ovf_ir)
                o_s = np.concatenate(ovf_s1024)
                o_w = np.concatenate(ovf_w)
                oo = np.argsort(o_ir, kind="stable")
                o_ir, o_s, o_w = o_ir[oo], o_s[oo], o_w[oo]
            else:
                o_ir = np.zeros(0, np.int64)
                o_s = np.zeros(0, np.int64)
                o_w = np.zeros(0, np.float32)
            novf = len(o_ir)
            if novf > c_.OVF_COLS * 128:
                raise OverflowError(f"core {c} gj {gj}: ovf {novf} > {c_.OVF_COLS * 128}")
            ob = io + nc_gj * 128
            pos = np.arange(novf)
            idx_flat[ob + pos] = o_ir
            slot_t[pos % 128, so + nc_gj + pos // 128] = o_s
            w_t[pos % 128, so + nc_gj + pos // 128] = o_w

        # wrap idx per gj: flat i -> [i % 16, i // 16], then concat, then
        # replicate across the 8 groups of 16 partitions.
        parts = []
        for gj in range(c_.NGJ):
            a = idx_flat[idx_off[gj]:idx_off[gj + 1]]
            parts.append(a.reshape(-1, 16).T)
        wr = np.concatenate(parts, axis=1)  # [16, tot_idx // 16]
        idx_dev = np.tile(wr, (8, 1)).astype(np.int16)
        cores.append(dict(idx=idx_dev, slot=slot_t, w=w_t))
    return structure, cores


def b_c_ln(arr, m):
    return arr[m]


def _bf16(a):
    import ml_dtypes
    return np.asarray(a, dtype=ml_dtypes.bfloat16)


# ---------------- device program ----------------
def build_program(structure):
    c_ = CFG
    ncols = structure["ncols"]
    col2b = structure["col2b"]
    num_idxs = structure["num_idxs"]
    idx_off = structure["idx_off"]
    sc_off = structure["sc_off"]
    tot_idx = structure["tot_idx"]
    tot_sc = structure["tot_sc"]

    nc = bacc.Bacc("TRN2", target_bir_lowering=False, debug=False,
                   enable_asserts=True, num_devices=c_.NC, num_swdge_queues=4)
    bf = mybir.dt.bfloat16
    f16 = mybir.dt.float16
    f32 = mybir.dt.float32
    x_t = nc.dram_tensor("x", [c_.NLOC, c_.F_IN], bf, kind="ExternalInput").ap()
    idx_t = nc.dram_tensor("idx", [128, tot_idx // 16], mybir.dt.int16, kind="ExternalInput").ap()
    slot_td = nc.dram_tensor("slot", [128, tot_sc], f32, kind="ExternalInput").ap()
    w_td = nc.dram_tensor("wtab", [128, tot_sc], f32, kind="ExternalInput").ap()
    iota_td = nc.dram_tensor("iota", [128, c_.GRP], f32, kind="ExternalInput").ap()
    w0_t = nc.dram_tensor("W0", [c_.F_IN, c_.H], bf, kind="ExternalInput").ap()
    wt_t = nc.dram_tensor("Wt", [c_.L, c_.H, c_.H], bf, kind="ExternalInput").ap()
    w1_t = nc.dram_tensor("W1", [c_.H, c_.C], bf, kind="ExternalInput").ap()
    out_t = nc.dram_tensor("out", [c_.NLOC, c_.C], f32, kind="ExternalOutput").ap()

    slice_b = nc.dram_tensor("slice_b", [c_.NLOC, c_.H], bf, kind="Internal").ap()
    replica = nc.dram_tensor("replica", [c_.NC * c_.NLOC, c_.H], bf, kind="Internal",
                             addr_space="Shared").ap()
    rg = [list(range(c_.NC))]

    with tile.TileContext(nc) as tc:
        with tc.tile_pool(name="res", bufs=1) as res, \
             tc.tile_pool(name="work", bufs=1) as work, \
             tc.tile_pool(name="psum", bufs=1, space="PSUM") as psum:
            # resident tensors
            idx = res.tile([128, tot_idx // 16], mybir.dt.int16)
            nc.sync.dma_start(idx[:], idx_t[:])
            slot_s = res.tile([128, tot_sc], f32)
            nc.sync.dma_start(slot_s[:], slot_td[:])
            slot_w = res.tile([128, tot_sc], f32)
            nc.sync.dma_start(slot_w[:], w_td[:])
            iota = res.tile([128, c_.GRP], f32)
            nc.sync.dma_start(iota[:], iota_td[:])
            w0 = res.tile([128, c_.KT, c_.H], bf)
            nc.sync.dma_start(w0[:], w0_t[:].rearrange("(k p) f -> p k f", p=128))
            wt = res.tile([128, c_.L, c_.H], bf)
            nc.sync.dma_start(wt[:], wt_t[:].rearrange("l f j -> f l j"))
            w1 = res.tile([128, c_.C], bf)
            nc.sync.dma_start(w1[:], w1_t[:])
            ident = res.tile([128, 128], bf)
            make_identity(nc, ident[:])
            x0rows = res.tile([128, c_.NBLK_TOT * 128], bf)

            # zero the gather buffers once (stale data is multiplied by
            # zero weights; it must be finite, not uninitialized SBUF)
            MAXCOLS = max(ncols) + c_.OVF_COLS
            for q in range(4):
                gtz = work.tile([128, MAXCOLS, 128], bf, name=f"gt{q}", tag="gt", bufs=4)
                nc.vector.memset(gtz[:], 0.0)

            # ---------- layer 0: h0 = relu(x @ W0); x0rows = alpha*h0 ----------
            ctx0 = nc.named_scope("layer0"); ctx0.__enter__()
            for g in range(c_.NGRP):
                ph0 = psum.tile([128, c_.GRP], f32, name=f"pbig{g%2}", tag="pbig", bufs=2, space="PSUM")
                for k in range(c_.KT):
                    xt = work.tile([128, c_.GRP], bf, name=f"xt{k%3}", tag="xt", bufs=3)
                    nc.sync.dma_start(out=xt[:], in_=x_t[g * c_.GRP:(g + 1) * c_.GRP, k * 128:(k + 1) * 128], transpose=True)
                    for hh in range(2):
                        nc.tensor.matmul(out=ph0[:, hh * 512:(hh + 1) * 512],
                                         lhsT=w0[:, k, :], rhs=xt[:, hh * 512:(hh + 1) * 512],
                                         start=(k == 0), stop=(k == c_.KT - 1))
                h0T = work.tile([128, c_.GRP], bf, name=f"h0T{g%2}", tag="h0T", bufs=2)
                nc.scalar.activation(out=h0T[:, :512], in_=ph0[:, :512], func=mybir.ActivationFunctionType.Relu)
                nc.scalar.activation(out=h0T[:, 512:], in_=ph0[:, 512:], func=mybir.ActivationFunctionType.Relu)
                for b in range(c_.NBLK):
                    gb = g * c_.NBLK + b
                    ptr = psum.tile([128, 128], bf, name=f"ptr{b%3}", tag="p2", bufs=4, space="PSUM")
                    nc.tensor.transpose(out=ptr[:], in_=h0T[:, b * 128:(b + 1) * 128], identity=ident[:])
                    hrow = work.tile([128, 128], bf, name=f"hrow{b%2}", tag="hrow", bufs=4)
                    nc.vector.tensor_copy(out=hrow[:], in_=ptr[:])
                    nc.sync.dma_start(out=slice_b[g * c_.GRP + b * 128: g * c_.GRP + (b + 1) * 128, :], in_=hrow[:])
                    nc.scalar.activation(out=x0rows[:, gb * 128:(gb + 1) * 128], in_=ptr[:],
                                         func=mybir.ActivationFunctionType.Relu, scale=c_.ALPHA)
            ctx0.__exit__(None, None, None)
            ctxag = nc.named_scope("ag0"); ctxag.__enter__()
            nc.gpsimd.collective_compute(
                "AllGather", mybir.AluOpType.bypass, replica_groups=rg,
                ins=[slice_b[:]], outs=[replica[:]])
            ctxag.__exit__(None, None, None)

            # ---------- conv layers ----------
            for l in range(c_.L):
                last = (l == c_.L - 1)
                ctxl = nc.named_scope(f"conv{l}"); ctxl.__enter__()
                for g in range(c_.NGRP):
                    pagg = psum.tile([128, c_.GRP], f32, name=f"pbig{g%2}", tag="pbig", bufs=2, space="PSUM")
                    # x0 residual doubles as the PSUM init for every block
                    for b in range(c_.NBLK):
                        gb = g * c_.NBLK + b
                        nc.tensor.matmul(out=pagg[:, b * 128:(b + 1) * 128],
                                         lhsT=x0rows[:, gb * 128:(gb + 1) * 128], rhs=ident[:],
                                         start=True, stop=False, skip_group_check=True)
                    for j2 in range(c_.NBUCK):
                        gj = g * c_.NBUCK + j2
                        ncg = ncols[gj]
                        ntot = ncg + c_.OVF_COLS
                        gt = work.tile([128, MAXCOLS, 128], bf, name=f"gt{j2}", tag="gt", bufs=4)
                        nc.gpsimd.dma_gather(
                            out_ap=gt[:, :ntot, :],
                            in_ap=replica[j2 * c_.WIN:(j2 + 1) * c_.WIN, :],
                            idxs_ap=idx[:, int(idx_off[gj]) // 16: int(idx_off[gj + 1]) // 16],
                            num_idxs=num_idxs[gj], num_idxs_reg=num_idxs[gj],
                            elem_size=c_.H, queue_num=j2, single_packet=False)
                        so_base = int(sc_off[gj])
                        sm = work.tile([128, MAXCOLS * 128], bf, name=f"sm{j2%3}", tag="sm", bufs=3)
                        for k in range(ncg):
                            nc.vector.tensor_scalar(
                                out=sm[:, k * 128:(k + 1) * 128], in0=iota[:, :128],
                                scalar1=slot_s[:, so_base + k: so_base + k + 1],
                                scalar2=slot_w[:, so_base + k: so_base + k + 1],
                                op0=mybir.AluOpType.is_equal, op1=mybir.AluOpType.mult)
                        so = work.tile([128, c_.OVF_COLS * c_.GRP], bf, name=f"so{j2%3}", tag="so", bufs=3)
                        for oc in range(c_.OVF_COLS):
                            nc.vector.tensor_scalar(
                                out=so[:, oc * c_.GRP:(oc + 1) * c_.GRP],
                                in0=iota[:],
                                scalar1=slot_s[:, so_base + ncg + oc: so_base + ncg + oc + 1],
                                scalar2=slot_w[:, so_base + ncg + oc: so_base + ncg + oc + 1],
                                op0=mybir.AluOpType.is_equal, op1=mybir.AluOpType.mult)
                        for k in range(ncg):
                            b = col2b[gj][k]
                            nc.tensor.matmul(out=pagg[:, b * 128:(b + 1) * 128],
                                             lhsT=gt[:, k, :], rhs=sm[:, k * 128:(k + 1) * 128],
                                             start=False, stop=False, skip_group_check=True)
                        lastj = (j2 == c_.NBUCK - 1)
                        for oc in range(c_.OVF_COLS):
                            for hh in range(2):
                                nc.tensor.matmul(
                                    out=pagg[:, hh * 512:(hh + 1) * 512],
                                    lhsT=gt[:, ncg + oc, :],
                                    rhs=so[:, oc * c_.GRP + hh * 512: oc * c_.GRP + (hh + 1) * 512],
                                    start=False,
                                    stop=(lastj and oc == c_.OVF_COLS - 1),
                                    skip_group_check=True)
                    outT = work.tile([128, c_.GRP], bf, name=f"outT{g%2}", tag="outT", bufs=2)
                    nc.vector.tensor_copy(out=outT[:, :512], in_=pagg[:, :512])
                    nc.vector.tensor_copy(out=outT[:, 512:], in_=pagg[:, 512:])
                    for b in range(c_.NBLK):
                        if not last:
                            p2 = psum.tile([128, 128], f32, name=f"p2{b%3}", tag="p2", bufs=4, space="PSUM")
                            nc.tensor.matmul(out=p2[:], lhsT=outT[:, b * 128:(b + 1) * 128],
                                             rhs=wt[:, l, :], start=True, stop=True)
                            hnew = work.tile([128, 128], bf, name=f"hnew{b%2}", tag="hrow", bufs=4)
                            nc.scalar.activation(out=hnew[:], in_=p2[:], func=mybir.ActivationFunctionType.Relu)
                            eng = nc.sync if b % 2 == 0 else nc.scalar
                            eng.dma_start(out=slice_b[g * c_.GRP + b * 128: g * c_.GRP + (b + 1) * 128, :], in_=hnew[:])
                        else:
                            p2 = psum.tile([128, 128], f32, name=f"p2{b%3}", tag="p2", bufs=4, space="PSUM")
                            nc.tensor.matmul(out=p2[:], lhsT=wt[:, l, :],
                                             rhs=outT[:, b * 128:(b + 1) * 128], start=True, stop=True)
                            h8T = work.tile([128, 128], bf, name=f"h8T{b%2}", tag="hrow", bufs=4)
                            nc.scalar.activation(out=h8T[:], in_=p2[:], func=mybir.ActivationFunctionType.Relu)
                            plg = psum.tile([128, 128], f32, name=f"plg{b%3}", tag="p2", bufs=4, space="PSUM")
                            nc.tensor.matmul(out=plg[:, :c_.C], lhsT=h8T[:], rhs=w1[:], start=True, stop=True)
                            negm = work.tile([128, 1], f32, name=f"negm{b%2}", tag="negm", bufs=4)
                            nc.vector.reduce_max(out=negm[:], in_=plg[:, :c_.C], axis=mybir.AxisListType.X, negate=True)
                            esc = work.tile([128, c_.C], bf, name=f"esc{b%2}", tag="esc", bufs=2)
                            ssum = work.tile([128, 1], f32, name=f"ssum{b%2}", tag="ssum", bufs=4)
                            nc.scalar.activation(out=esc[:], in_=plg[:, :c_.C], func=mybir.ActivationFunctionType.Exp,
                                                 bias=negm[:, :1], accum_out=ssum[:, :1])
                            lsum = work.tile([128, 1], f32, name=f"lsum{b%2}", tag="lsum", bufs=4)
                            nc.scalar.activation(out=lsum[:], in_=ssum[:], func=mybir.ActivationFunctionType.Ln)
                            fin = work.tile([128, c_.C], f32, name=f"fin{b%2}", tag="fin", bufs=4)
                            nc.vector.tensor_scalar(out=fin[:], in0=plg[:, :c_.C],
                                                    scalar1=negm[:, :1], scalar2=lsum[:, :1],
                                                    op0=mybir.AluOpType.add, op1=mybir.AluOpType.subtract)
                            eng = nc.sync if b % 2 == 0 else nc.scalar
                            eng.dma_start(out=out_t[g * c_.GRP + b * 128: g * c_.GRP + (b + 1) * 128, :], in_=fin[:])
                ctxl.__exit__(None, None, None)
                if not last:
                    ctxa = nc.named_scope(f"ag{l+1}"); ctxa.__enter__()
                    nc.gpsimd.collective_compute(
                        "AllGather", mybir.AluOpType.bypass, replica_groups=rg,
                        ins=[slice_b[:]], outs=[replica[:]])
                    ctxa.__exit__(None, None, None)
    nc.compile()
    return nc


# ---------------- end-to-end host entry ----------------
_CACHED = {}


def kernel_ex(x, edge_index, edge_weight, W0, b0, convW, W1, b1, trace=False):
    c_ = CFG
    x = np.asarray(x); edge_index = np.asarray(edge_index); edge_weight = np.asarray(edge_weight)
    W0 = np.asarray(W0); convW = np.asarray(convW); W1 = np.asarray(W1)
    b0 = np.asarray(b0); b1 = np.asarray(b1)
    assert np.abs(b0).max() == 0.0 and np.abs(b1).max() == 0.0, "nonzero biases unsupported"
    while True:
        try:
            structure, cores = preprocess(edge_index, edge_weight)
            break
        except OverflowError:
            CFG.OVF_COLS += 1
            CFG.derived()
            _CACHED.pop("nc", None)
    betas = np.log(c_.THETA / np.arange(1, c_.L + 1, dtype=np.float64) + 1.0)
    Wt = np.stack([(1 - bt) * np.eye(c_.H) + bt * Wl.astype(np.float64)
                   for Wl, bt in zip(convW, betas)]).astype(np.float32)
    if "nc" not in _CACHED:
        _CACHED["nc"] = build_program(structure)
    nc = _CACHED["nc"]
    iota = np.tile(np.arange(c_.GRP, dtype=np.float32), (128, 1))
    in_maps = []
    for c in range(c_.NC):
        xs = np.zeros((c_.NLOC, c_.F_IN), np.float32)
        xs[:c_.NPC] = x[c * c_.NPC:(c + 1) * c_.NPC]
        in_maps.append({
            "x": _bf16(xs), "idx": cores[c]["idx"],
            "slot": cores[c]["slot"], "wtab": cores[c]["w"], "iota": iota,
            "W0": _bf16(W0), "Wt": _bf16(Wt), "W1": _bf16(W1),
        })
    res = bass_utils.run_bass_kernel_spmd(nc, in_maps, core_ids=list(range(c_.NC)), trace=trace)
    out = np.concatenate([res.results[c]["out"][:c_.NPC] for c in range(c_.NC)], axis=0)
    return out, res


def kernel(x, edge_index, edge_weight, W0, b0, convW, W1, b1):
    """Harness entry: full inputs in, full [N, C] float32 log-softmax out."""
    out, _ = kernel_ex(x, edge_index, edge_weight, W0, b0, convW, W1, b1, trace=False)
    return out
